# revision 3
# baseline (speedup 1.0000x reference)
"""Trainium2 Bass kernel for masked attention-pooling (DmasifAttentionModule).

Reference computation (per sample b):
    proj   = x @ W.T + b                  # [N, D]
    scores = proj @ v                     # [N]
    scores = where(mask, scores, -1e9)
    w      = softmax(scores)              # [N]
    out    = w @ x                        # [D]

Algebraic collapse used here (exact up to fp reassociation):
    scores = x @ (W.T @ v) + (b . v)
and softmax is shift-invariant, so the (b . v) constant drops out entirely.
With u = v @ W (a 512-vector computed on host), the device work is:
    s[n]  = sum_d x[n,d] * u[d]
    e     = exp(s + mb[n])                    (mb = -C for valid rows for exp
                                               range safety, -3e8 for masked
                                               rows -> exp underflows to 0)
    Z     = sum_n e[n]
    out   = (sum_n e[n] * x[n,:]) / Z

Per-core layout (8 cores, 2 samples each, data-parallel over batch):
    - x shard [2, 4096, 512] f32 streamed as 16 x 1MiB tiles [128, 4, 512]
      (partition = n%128, free = (n//128 % 4, d)); tiles stay resident in SBUF.
    - scores: DVE scalar_tensor_tensor (fused mul + free-dim accum-reduce).
      (tensor_tensor_reduce would fold the mask into the reduce init, but that
      opcode hard-crashes this runtime - NRT_EXEC_UNIT_UNRECOVERABLE.)
    - exp: ScalarE activation, one column per op so the mask/shift column
      rides the per-partition bias operand.
    - pooling + Z: TensorE matvec accumulation into PSUM
      (lhsT = e column [128,1], rhs = x chunk [128,512]).
This is HBM-bandwidth bound: 16 MiB/core read once (~47 us at ~358 GB/s).
"""

import os
import sys

import numpy as np

for _p in ("/opt/trn_rl_repo", "/root/.axon_site/_ro/trn_rl_repo"):
    if os.path.isdir(_p) and _p not in sys.path:
        sys.path.append(_p)

import concourse.bacc as bacc
import concourse.tile as tile
from concourse import mybir
from concourse.bass_utils import run_bass_kernel_spmd

B, N, D = 16, 4096, 512
N_CORES = 8
SPB = B // N_CORES          # samples per core
TILES = 8                   # 1MiB x-tiles per sample
COLS = N // 128             # 32 score columns of 128 n's per sample
CPT = COLS // TILES         # score columns per tile (4)
C_SHIFT = 24.0              # constant exp-range shift (softmax-invariant)
MASKED_INIT = -3.0e8        # masked scores -> exp underflows to exactly 0

_F32 = mybir.dt.float32
_CACHE = {}


def _build_program():
    nc = bacc.Bacc("TRN2", target_bir_lowering=False, debug=False)
    x = nc.dram_tensor("x", [SPB, N, D], _F32, kind="ExternalInput").ap()
    mb = nc.dram_tensor("mb", [SPB, 128, COLS], _F32, kind="ExternalInput").ap()
    u = nc.dram_tensor("u", [128, D], _F32, kind="ExternalInput").ap()
    out = nc.dram_tensor("out", [SPB, D], _F32, kind="ExternalOutput").ap()

    # [s, i, p, c, d]: n = i*512 + c*128 + p
    x5 = x.rearrange("s (i c p) d -> s i p c d", i=TILES, c=CPT, p=128)

    with tile.TileContext(nc) as tc:
        with (
            tc.tile_pool(name="xp", bufs=SPB * TILES) as xp,
            tc.tile_pool(name="singles", bufs=1) as sg,
            tc.tile_pool(name="scratch", bufs=4) as scr,
            tc.tile_pool(name="smalls", bufs=2 * SPB) as sm,
            tc.tile_pool(name="ps", bufs=2 * SPB, space="PSUM") as psp,
        ):
            ones_sb = sg.tile([128, 1], _F32)
            nc.vector.memset(ones_sb[:], 1.0)
            warm = sg.tile([128, 1], _F32)
            # Pull the exp table-set load (~2.7us) to t=0, under the DMAs.
            nc.scalar.activation(warm[:], ones_sb[:],
                                 mybir.ActivationFunctionType.Exp)

            u_sb = sg.tile([128, D], _F32)
            nc.sync.dma_start(out=u_sb[:], in_=u[:])
            mb_sb = sg.tile([128, SPB, COLS], _F32)
            nc.sync.dma_start(out=mb_sb[:], in_=mb.rearrange("s p c -> p s c"))

            x_tiles = {}
            for s in range(SPB):
                for i in range(TILES):
                    t = xp.tile([128, CPT, D], _F32)
                    nc.sync.dma_start(out=t[:], in_=x5[s, i])
                    x_tiles[(s, i)] = t

            s_sb = sg.tile([128, SPB, COLS], _F32)
            e_sb = sg.tile([128, SPB, COLS], _F32)
            zc_sb = sg.tile([128, SPB], _F32)

            for s in range(SPB):
                pool_ps = psp.tile([1, D], _F32)
                z_ps = psp.tile([1, 1], _F32)
                for i in range(TILES):
                    xt = x_tiles[(s, i)]
                    for c in range(CPT):
                        col = i * CPT + c
                        dump = scr.tile([128, 1], _F32)
                        nc.vector.scalar_tensor_tensor(
                            out=dump.broadcast_to((128, D)),
                            in0=xt[:, c, :],
                            scalar=1.0,
                            in1=u_sb[:],
                            op0=mybir.AluOpType.mult,
                            op1=mybir.AluOpType.mult,
                            accum_out=s_sb[:, s, col:col + 1],
                        )
                        # e = exp(s - C) valid rows, exp(s - 3e8) = 0 masked
                        nc.scalar.activation(
                            e_sb[:, s, col:col + 1], s_sb[:, s, col:col + 1],
                            mybir.ActivationFunctionType.Exp,
                            bias=mb_sb[:, s, col:col + 1])
                    for c in range(CPT):
                        col = i * CPT + c
                        nc.tensor.matmul(
                            pool_ps[:],
                            e_sb[:, s, col:col + 1],
                            xt[:, c, :],
                            start=(i == 0 and c == 0),
                            stop=(i == TILES - 1 and c == CPT - 1),
                        )
                nc.vector.tensor_reduce(
                    zc_sb[:, s:s + 1], e_sb[:, s, :],
                    axis=mybir.AxisListType.X, op=mybir.AluOpType.add)
                nc.tensor.matmul(z_ps[:], ones_sb[:], zc_sb[:, s:s + 1],
                                 start=True, stop=True)
                zi_sb = sm.tile([1, 1], _F32)
                nc.vector.reciprocal(zi_sb[:], z_ps[:])
                o_sb = sm.tile([1, D], _F32)
                nc.scalar.activation(o_sb[:], pool_ps[:],
                                     mybir.ActivationFunctionType.Copy,
                                     scale=zi_sb[:])
                nc.sync.dma_start(out=out[s:s + 1, :], in_=o_sb[:])

    nc.compile()
    return nc


def _get_program():
    if "nc" not in _CACHE:
        _CACHE["nc"] = _build_program()
    return _CACHE["nc"]


def kernel(x, flat_mask, W, b, v, **_unused):
    x = np.ascontiguousarray(x, dtype=np.float32)
    W = np.asarray(W, dtype=np.float32)
    v = np.asarray(v, dtype=np.float32)
    # scores = x @ u + (b . v); the constant is dropped by softmax invariance.
    u = (v @ W).astype(np.float32)
    u_rep = np.ascontiguousarray(np.broadcast_to(u, (128, D)), dtype=np.float32)

    # Reduction-init column: -C_SHIFT for valid rows, very negative for masked.
    mb = np.where(np.asarray(flat_mask) == 1,
                  np.float32(-C_SHIFT), np.float32(MASKED_INIT))
    # [B, N] -> [B, 128, COLS] with [b, p, col] <- n = col*128 + p
    mb = np.ascontiguousarray(
        mb.reshape(B, COLS, 128).transpose(0, 2, 1).astype(np.float32))

    in_maps = []
    for core in range(N_CORES):
        lo = core * SPB
        in_maps.append({
            "x": np.ascontiguousarray(x[lo:lo + SPB]),
            "mb": np.ascontiguousarray(mb[lo:lo + SPB]),
            "u": u_rep,
        })

    nc = _get_program()
    res = run_bass_kernel_spmd(nc, in_maps, core_ids=list(range(N_CORES)))
    return np.concatenate([res.results[i]["out"] for i in range(N_CORES)],
                          axis=0)


# revision 7
# speedup vs baseline: 14009.3787x; 14009.3787x over previous
"""Trainium2 Bass kernel for masked attention-pooling (DmasifAttentionModule).

Reference computation (per sample b):
    proj   = x @ W.T + b                  # [N, D]
    scores = proj @ v                     # [N]
    scores = where(mask, scores, -1e9)
    w      = softmax(scores)              # [N]
    out    = w @ x                        # [D]

Algebraic collapse used here (exact up to fp reassociation):
    scores = x @ (W.T @ v) + (b . v)
and softmax is shift-invariant, so the (b . v) constant drops out entirely.
With u = v @ W (a 512-vector computed on host), the device work is:
    s[n]  = sum_d x[n,d] * u[d]
    e     = exp(s + mb[n])                    (mb = -C for valid rows for exp
                                               range safety, -3e8 for masked
                                               rows -> exp underflows to 0)
    Z     = sum_n e[n]
    out   = (sum_n e[n] * x[n,:]) / Z

Per-core layout (8 cores, 2 samples each, data-parallel over batch):
    - x shard [2, 4096, 512] f32 streamed as 16 x 1MiB tiles [128, 4, 512]
      (partition = n%128, free = (n//128 % 4, d)); tiles stay resident in SBUF.
    - scores: DVE scalar_tensor_tensor (fused mul + free-dim accum-reduce).
      (tensor_tensor_reduce would fold the mask into the reduce init, but that
      opcode hard-crashes this runtime - NRT_EXEC_UNIT_UNRECOVERABLE.)
    - exp: ScalarE activation, one column per op so the mask/shift column
      rides the per-partition bias operand.
    - pooling + Z: TensorE matvec accumulation into PSUM
      (lhsT = e column [128,1], rhs = x chunk [128,512]).
This is HBM-bandwidth bound: 16 MiB/core read once (~47 us at ~358 GB/s).
"""

import os
import sys

import numpy as np

for _p in ("/opt/trn_rl_repo", "/root/.axon_site/_ro/trn_rl_repo"):
    if os.path.isdir(_p) and _p not in sys.path:
        sys.path.append(_p)

import concourse.bacc as bacc
import concourse.tile as tile
from concourse import mybir
from concourse.bass_utils import run_bass_kernel_spmd

B, N, D = 16, 4096, 512
N_CORES = 8
SPB = B // N_CORES          # samples per core
TILES = 8                   # 1MiB x-tiles per sample
COLS = N // 128             # 32 score columns of 128 n's per sample
CPT = COLS // TILES         # score columns per tile (4)
C_SHIFT = 24.0              # constant exp-range shift (softmax-invariant)
MASKED_INIT = -3.0e8        # masked scores -> exp underflows to exactly 0

_F32 = mybir.dt.float32
_CACHE = {}


def _build_program(unroll=1):
    """unroll>1 repeats the whole computation in one NEFF (timing only)."""
    nc = bacc.Bacc("TRN2", target_bir_lowering=False, debug=False)
    x = nc.dram_tensor("x", [SPB, N, D], _F32, kind="ExternalInput").ap()
    mb = nc.dram_tensor("mb", [SPB, 128, COLS], _F32, kind="ExternalInput").ap()
    u = nc.dram_tensor("u", [128, D], _F32, kind="ExternalInput").ap()
    out = nc.dram_tensor("out", [SPB, D], _F32, kind="ExternalOutput").ap()

    # [s, i, p, c, d]: n = i*512 + c*128 + p
    x5 = x.rearrange("s (i c p) d -> s i p c d", i=TILES, c=CPT, p=128)

    with tile.TileContext(nc) as tc:
        with (
            tc.tile_pool(name="xp", bufs=SPB * TILES) as xp,
            tc.tile_pool(name="singles", bufs=1) as sg,
            tc.tile_pool(name="scratch", bufs=4) as scr,
            tc.tile_pool(name="smalls", bufs=2) as sm,
            tc.tile_pool(name="ps", bufs=2, space="PSUM") as psp,
        ):
            ones_sb = sg.tile([128, 1], _F32)
            nc.vector.memset(ones_sb[:], 1.0)
            warm = sg.tile([128, 1], _F32)
            # Pull the exp table-set load (~2.7us) to t=0, under the DMAs.
            nc.scalar.activation(warm[:], ones_sb[:],
                                 mybir.ActivationFunctionType.Exp)

            u_sb = sg.tile([128, D], _F32)
            nc.sync.dma_start(out=u_sb[:], in_=u[:])
            mb_sb = sg.tile([128, SPB, COLS], _F32)
            nc.sync.dma_start(out=mb_sb[:], in_=mb.rearrange("s p c -> p s c"))

            s_sb = sg.tile([128, SPB, COLS], _F32)
            e_sb = sg.tile([128, SPB, COLS], _F32)
            zc_sb = sg.tile([128, SPB], _F32)

            for _it in range(unroll):
                _emit_iteration(nc, tc, xp, scr, sm, psp, x5, out,
                                u_sb, mb_sb, ones_sb, s_sb, e_sb, zc_sb)

    nc.compile()
    return nc


def _emit_iteration(nc, tc, xp, scr, sm, psp, x5, out,
                    u_sb, mb_sb, ones_sb, s_sb, e_sb, zc_sb):
    x_tiles = {}
    for s in range(SPB):
        for i in range(TILES):
            t = xp.tile([128, CPT, D], _F32, name=f"xt_{s}_{i}", bufs=1)
            nc.sync.dma_start(out=t[:], in_=x5[s, i])
            x_tiles[(s, i)] = t

    for s in range(SPB):
        pool_ps = psp.tile([1, D], _F32, name=f"pool_ps_{s}")
        z_ps = psp.tile([1, 1], _F32, name=f"z_ps_{s}")
        for i in range(TILES):
            xt = x_tiles[(s, i)]
            for c in range(CPT):
                col = i * CPT + c
                dump = scr.tile([128, 1], _F32, name="dump")
                nc.vector.scalar_tensor_tensor(
                    out=dump.broadcast_to((128, D)),
                    in0=xt[:, c, :],
                    scalar=1.0,
                    in1=u_sb[:],
                    op0=mybir.AluOpType.mult,
                    op1=mybir.AluOpType.mult,
                    accum_out=s_sb[:, s, col:col + 1],
                )
                # e = exp(s - C) valid rows, exp(s - 3e8) = 0 masked
                nc.scalar.activation(
                    e_sb[:, s, col:col + 1], s_sb[:, s, col:col + 1],
                    mybir.ActivationFunctionType.Exp,
                    bias=mb_sb[:, s, col:col + 1])
            for c in range(CPT):
                col = i * CPT + c
                nc.tensor.matmul(
                    pool_ps[:],
                    e_sb[:, s, col:col + 1],
                    xt[:, c, :],
                    start=(i == 0 and c == 0),
                    stop=(i == TILES - 1 and c == CPT - 1),
                )
        nc.vector.tensor_reduce(
            zc_sb[:, s:s + 1], e_sb[:, s, :],
            axis=mybir.AxisListType.X, op=mybir.AluOpType.add)
        nc.tensor.matmul(z_ps[:], ones_sb[:], zc_sb[:, s:s + 1],
                         start=True, stop=True)
        zi_sb = sm.tile([1, 1], _F32, name=f"zi_{s}")
        nc.vector.reciprocal(zi_sb[:], z_ps[:])
        o_sb = sm.tile([1, D], _F32, name=f"o_{s}")
        nc.scalar.activation(o_sb[:], pool_ps[:],
                             mybir.ActivationFunctionType.Copy,
                             scale=zi_sb[:])
        nc.sync.dma_start(out=out[s:s + 1, :], in_=o_sb[:])


def _get_program():
    if "nc" not in _CACHE:
        _CACHE["nc"] = _build_program()
    return _CACHE["nc"]


def kernel(x, flat_mask, W, b, v, **_unused):
    x = np.ascontiguousarray(x, dtype=np.float32)
    W = np.asarray(W, dtype=np.float32)
    v = np.asarray(v, dtype=np.float32)
    # scores = x @ u + (b . v); the constant is dropped by softmax invariance.
    u = (v @ W).astype(np.float32)
    u_rep = np.ascontiguousarray(np.broadcast_to(u, (128, D)), dtype=np.float32)

    # Reduction-init column: -C_SHIFT for valid rows, very negative for masked.
    mb = np.where(np.asarray(flat_mask) == 1,
                  np.float32(-C_SHIFT), np.float32(MASKED_INIT))
    # [B, N] -> [B, 128, COLS] with [b, p, col] <- n = col*128 + p
    mb = np.ascontiguousarray(
        mb.reshape(B, COLS, 128).transpose(0, 2, 1).astype(np.float32))

    in_maps = []
    for core in range(N_CORES):
        lo = core * SPB
        in_maps.append({
            "x": np.ascontiguousarray(x[lo:lo + SPB]),
            "mb": np.ascontiguousarray(mb[lo:lo + SPB]),
            "u": u_rep,
        })

    nc = _get_program()
    res = run_bass_kernel_spmd(nc, in_maps, core_ids=list(range(N_CORES)))
    return np.concatenate([res.results[i]["out"] for i in range(N_CORES)],
                          axis=0)
